# revision 5
# baseline (speedup 1.0000x reference)
"""HardMemory retrieval-KNN kernel for 8 Trainium2 NeuronCores.

Data-parallel: 32 batches sharded 4-per-core; memory bank [1024,512]
replicated. Per batch b (x_b = [C=512, N=4096] f32):
  sim[n,m]   = <x_n, mem_m / ||mem_m||>          (bf16 matmul, fp32 accum)
  sumsq[n]   = ||x_n||^2  (diag of X^T X, same stationary as sim matmul)
  mask[n]    = max_m sim > 0.8 * sqrt(sumsq)      (== cos > 0.8)
  onehot[n,m]= (sim == rowmax) * mask             (bf16 {0,1})
  out[:, n]  = memory^T @ onehot^T                (masked gather as matmul)
"""

import sys

for _p in ("/opt/trn_rl_repo",):
    if _p not in sys.path:
        sys.path.insert(0, _p)

from contextlib import ExitStack

import ml_dtypes
import numpy as np

import concourse.bass as bass
import concourse.tile as tile
from concourse import bacc, mybir
from concourse.bass_utils import run_bass_kernel_spmd

F32 = mybir.dt.float32
BF16 = mybir.dt.bfloat16
AF = mybir.ActivationFunctionType
ALU = mybir.AluOpType

B_FULL, C, H, W = 32, 512, 64, 64
N_PIX = H * W            # pixels per batch
M = 1024                 # memory rows
N_CORES = 8
B_LOC = B_FULL // N_CORES
THRESH2 = 0.8 * 0.8      # compare max^2-style: thr = sqrt(0.64 * sumsq)

CC = C // 128            # 4 contraction chunks
MC = M // 128            # 8 memory chunks


def build_kernel(b_loc=B_LOC, n_pix=N_PIX):
    ns_count = n_pix // 512          # n-supers per batch
    nt_per_batch = n_pix // 128      # 128-pixel tiles per batch

    nc = bacc.Bacc("TRN2", target_bir_lowering=False, debug=False,
                   num_devices=N_CORES)
    xs = nc.dram_tensor("xs", [b_loc, C, n_pix], F32, kind="ExternalInput")
    mem = nc.dram_tensor("memory", [M, C], F32, kind="ExternalInput")
    ident_b = nc.dram_tensor("identity", [128, 128], BF16, kind="ExternalInput")
    ident_f = nc.dram_tensor("identf", [128, 128], F32, kind="ExternalInput")
    out = nc.dram_tensor("out", [b_loc, C, n_pix], F32, kind="ExternalOutput")

    with tile.TileContext(nc) as tc, ExitStack() as ctx:
        const = ctx.enter_context(tc.tile_pool(name="const", bufs=1))
        mstage = ctx.enter_context(tc.tile_pool(name="mstage", bufs=2))
        mtmp = ctx.enter_context(tc.tile_pool(name="mtmp", bufs=2))
        xio = ctx.enter_context(tc.tile_pool(name="xio", bufs=8))
        ohp = ctx.enter_context(tc.tile_pool(name="ohp", bufs=2))
        stats = ctx.enter_context(tc.tile_pool(name="stats", bufs=4))
        outb = ctx.enter_context(tc.tile_pool(name="outb", bufs=4))
        ohtp = ctx.enter_context(tc.tile_pool(name="ohtp", bufs=1))
        psumA = ctx.enter_context(
            tc.tile_pool(name="psumA", bufs=2, space=bass.MemorySpace.PSUM))
        psumX = ctx.enter_context(
            tc.tile_pool(name="psumX", bufs=1, space=bass.MemorySpace.PSUM))
        psumTR = ctx.enter_context(
            tc.tile_pool(name="psumTR", bufs=1, space=bass.MemorySpace.PSUM))
        psumB = ctx.enter_context(
            tc.tile_pool(name="psumB", bufs=2, space=bass.MemorySpace.PSUM))

        idb = const.tile([128, 128], BF16, tag="idb")
        nc.sync.dma_start(idb[:], ident_b[:])
        idf = const.tile([128, 128], F32, tag="idf")
        nc.sync.dma_start(idf[:], ident_f[:])

        # ---- memory preprocessing: norms, bf16 cast, transpose ----
        memS = [const.tile([128, C], BF16, tag=f"memS{mi}", name=f"memS{mi}")
                for mi in range(MC)]
        memT = [const.tile([128, M], BF16, tag=f"memT{ci}", name=f"memT{ci}")
                for ci in range(CC)]
        for mi in range(MC):
            mld = mstage.tile([128, C], F32)
            nc.sync.dma_start(mld[:], mem[mi * 128:(mi + 1) * 128, :])
            msq = mtmp.tile([128, C], F32, tag="msq")
            mssq = stats.tile([128, 1], F32, tag="mssq")
            nc.scalar.activation(msq[:], mld[:], AF.Square, accum_out=mssq[:])
            mnorm = stats.tile([128, 1], F32, tag="mnorm")
            nc.scalar.activation(mnorm[:], mssq[:], AF.Sqrt)
            rinv = stats.tile([128, 1], F32, tag="rinv")
            nc.vector.reciprocal(rinv[:], mnorm[:])
            nc.vector.tensor_copy(memS[mi][:], mld[:])
            mn = mtmp.tile([128, C], BF16, tag="mn")
            nc.vector.tensor_scalar_mul(mn[:], mld[:], rinv[:])
            for ci in range(CC):
                ptr = psumTR.tile([128, 128], BF16)
                nc.tensor.transpose(ptr[:], mn[:, ci * 128:(ci + 1) * 128], idb[:])
                nc.scalar.activation(
                    memT[ci][:, mi * 128:(mi + 1) * 128], ptr[:], AF.Copy)

        ohT = [ohtp.tile([128, n_pix], BF16, tag=f"ohT{mi}", name=f"ohT{mi}")
               for mi in range(MC)]

        for b in range(b_loc):
            # ---- phase A: similarity + argmax-onehot, transposed ----
            for ns in range(ns_count):
                xb = []
                for ci in range(CC):
                    xf = xio.tile([128, 512], F32, tag="xf")
                    nc.sync.dma_start(
                        xf[:],
                        xs[b, ci * 128:(ci + 1) * 128, ns * 512:(ns + 1) * 512])
                    xbc = xio.tile([128, 512], BF16, tag="xb")
                    nc.scalar.activation(xbc[:], xf[:], AF.Copy)
                    xb.append(xbc)
                for nt in range(4):
                    ntg = ns * 4 + nt          # global n-tile idx in batch
                    psA = psumA.tile([128, M], F32)
                    psX = psumX.tile([128, 128], F32)
                    for ci in range(CC):
                        st = xb[ci][:, nt * 128:(nt + 1) * 128]
                        nc.tensor.matmul(psA[:, 0:512], st, memT[ci][:, 0:512],
                                         start=(ci == 0), stop=(ci == CC - 1))
                        nc.tensor.matmul(psA[:, 512:1024], st,
                                         memT[ci][:, 512:1024],
                                         start=(ci == 0), stop=(ci == CC - 1))
                        nc.tensor.matmul(psX[:], st, st,
                                         start=(ci == 0), stop=(ci == CC - 1))
                    mx0 = stats.tile([128, 1], F32, tag="mx0")
                    nc.vector.tensor_reduce(mx0[:], psA[:, 0:512],
                                            mybir.AxisListType.X, ALU.max)
                    mx1 = stats.tile([128, 1], F32, tag="mx1")
                    nc.vector.tensor_reduce(mx1[:], psA[:, 512:1024],
                                            mybir.AxisListType.X, ALU.max)
                    mx = stats.tile([128, 1], F32, tag="mx")
                    nc.vector.tensor_tensor(mx[:], mx0[:], mx1[:], ALU.max)
                    dsq = mtmp.tile([128, 128], F32, tag="dsq")
                    nc.vector.tensor_tensor(dsq[:], psX[:], idf[:], ALU.mult)
                    ssq = stats.tile([128, 1], F32, tag="ssq")
                    nc.vector.tensor_reduce(ssq[:], dsq[:],
                                            mybir.AxisListType.X, ALU.add)
                    thr = stats.tile([128, 1], F32, tag="thr")
                    nc.scalar.activation(thr[:], ssq[:], AF.Sqrt, scale=THRESH2)
                    msk = stats.tile([128, 1], F32, tag="msk")
                    nc.vector.tensor_tensor(msk[:], mx[:], thr[:], ALU.is_gt)
                    oh = ohp.tile([128, M], BF16, tag="oh")
                    nc.vector.tensor_scalar(oh[:, 0:512], psA[:, 0:512],
                                            mx[:], msk[:],
                                            ALU.is_equal, ALU.mult)
                    nc.vector.tensor_scalar(oh[:, 512:1024], psA[:, 512:1024],
                                            mx[:], msk[:],
                                            ALU.is_equal, ALU.mult)
                    for mi in range(MC):
                        ptr = psumTR.tile([128, 128], BF16)
                        nc.tensor.transpose(
                            ptr[:], oh[:, mi * 128:(mi + 1) * 128], idb[:])
                        nc.scalar.activation(
                            ohT[mi][:, ntg * 128:(ntg + 1) * 128], ptr[:],
                            AF.Copy)
            # ---- phase B: out[:, n] = memory^T @ onehot^T ----
            for ci in range(CC):
                for ng in range(ns_count):
                    psB = psumB.tile([128, 512], F32)
                    for mi in range(MC):
                        nc.tensor.matmul(
                            psB[:], memS[mi][:, ci * 128:(ci + 1) * 128],
                            ohT[mi][:, ng * 512:(ng + 1) * 512],
                            start=(mi == 0), stop=(mi == MC - 1))
                    ob = outb.tile([128, 512], F32, tag="ob")
                    nc.scalar.activation(ob[:], psB[:], AF.Copy)
                    nc.sync.dma_start(
                        out[b, ci * 128:(ci + 1) * 128,
                            ng * 512:(ng + 1) * 512], ob[:])

    nc.compile()
    return nc


_NC_CACHE = {}


def _get_nc(b_loc=B_LOC, n_pix=N_PIX):
    key = (b_loc, n_pix)
    if key not in _NC_CACHE:
        _NC_CACHE[key] = build_kernel(*key)
    return _NC_CACHE[key]


def run_on_hw(x_flat, memory, b_loc=B_LOC, n_pix=N_PIX, trace=False,
              **spmd_kwargs):
    """x_flat: [N_CORES*b_loc, C, n_pix] f32. Returns (out_full, results)."""
    nc = _get_nc(b_loc, n_pix)
    ident_b = np.eye(128, dtype=ml_dtypes.bfloat16)
    ident_f = np.eye(128, dtype=np.float32)
    in_maps = [
        {
            "xs": np.ascontiguousarray(x_flat[c * b_loc:(c + 1) * b_loc]),
            "memory": memory,
            "identity": ident_b,
            "identf": ident_f,
        }
        for c in range(N_CORES)
    ]
    res = run_bass_kernel_spmd(nc, in_maps, list(range(N_CORES)),
                               trace=trace, **spmd_kwargs)
    outs = [res.results[c]["out"] for c in range(N_CORES)]
    return np.concatenate(outs, axis=0), res


def kernel(x, memory):
    x = np.asarray(x, dtype=np.float32)
    memory = np.asarray(memory, dtype=np.float32)
    B, C_, H_, W_ = x.shape
    x_flat = np.ascontiguousarray(x.reshape(B, C_, H_ * W_))
    out_flat, _ = run_on_hw(x_flat, memory)
    return out_flat.reshape(B, C_, H_, W_)


# revision 11
# speedup vs baseline: 1.7146x; 1.7146x over previous
"""HardMemory retrieval-KNN kernel for 8 Trainium2 NeuronCores.

Data-parallel: 32 batches sharded 4-per-core; memory bank [1024,512]
replicated. Per batch b (x_b = [C=512, N=4096] f32):
  simT[m,n]  = <x_n, mem_m / ||mem_m||>          (bf16 matmul, fp32 accum)
  sumsq[n]   = ||x_n||^2  (ones-stationary matmul over x^2)
  mx'[n]     = colmax(simT) + 1e30*(colmax <= 0.8*sqrt(sumsq))
  onehot[m,n]= (simT == bcast(mx'))               (bf16 {0,1}; mask folded in)
  out[:, n]  = memory^T @ onehot                  (masked gather as matmul)

simT layout avoids any onehot transpose; onehot feeds matmul2 directly
as the moving operand.
"""

import sys

for _p in ("/opt/trn_rl_repo",):
    if _p not in sys.path:
        sys.path.insert(0, _p)

from contextlib import ExitStack

import ml_dtypes
import numpy as np

import concourse.bass as bass
import concourse.tile as tile
from concourse import bacc, bass_isa, mybir
from concourse.bass_utils import run_bass_kernel_spmd

F32 = mybir.dt.float32
BF16 = mybir.dt.bfloat16
AF = mybir.ActivationFunctionType
ALU = mybir.AluOpType
AX = mybir.AxisListType

B_FULL, C, H, W = 32, 512, 64, 64
N_PIX = H * W
M = 1024
N_CORES = 8
B_LOC = B_FULL // N_CORES
THRESH2 = 0.8 * 0.8
BIG = 1.0e30

CC = C // 128            # 4 contraction chunks
MC = M // 128            # 8 memory chunks


def build_kernel(b_loc=B_LOC, n_pix=N_PIX):
    ns_count = n_pix // 512

    nc = bacc.Bacc("TRN2", target_bir_lowering=False, debug=False,
                   num_devices=N_CORES)
    xs = nc.dram_tensor("xs", [b_loc, C, n_pix], F32, kind="ExternalInput")
    mem = nc.dram_tensor("memory", [M, C], F32, kind="ExternalInput")
    ident_b = nc.dram_tensor("identity", [128, 128], BF16, kind="ExternalInput")
    out = nc.dram_tensor("out", [b_loc, C, n_pix], F32, kind="ExternalOutput")

    with tile.TileContext(nc) as tc, ExitStack() as ctx:
        const = ctx.enter_context(tc.tile_pool(name="const", bufs=1))
        mstage = ctx.enter_context(tc.tile_pool(name="mstage", bufs=2))
        mtmp = ctx.enter_context(tc.tile_pool(name="mtmp", bufs=2))
        xio = ctx.enter_context(tc.tile_pool(name="xio", bufs=16))
        simp = ctx.enter_context(tc.tile_pool(name="simp", bufs=2))
        stats = ctx.enter_context(tc.tile_pool(name="stats", bufs=4))
        outb = ctx.enter_context(tc.tile_pool(name="outb", bufs=4))
        psim = ctx.enter_context(
            tc.tile_pool(name="psim", bufs=4, space=bass.MemorySpace.PSUM))
        psq = ctx.enter_context(
            tc.tile_pool(name="psq", bufs=1, space=bass.MemorySpace.PSUM))
        pbc = ctx.enter_context(
            tc.tile_pool(name="pbc", bufs=1, space=bass.MemorySpace.PSUM))
        psumB = ctx.enter_context(
            tc.tile_pool(name="psumB", bufs=2, space=bass.MemorySpace.PSUM))

        idb = const.tile([128, 128], BF16, tag="idb")
        nc.sync.dma_start(idb[:], ident_b[:])
        ones_c = const.tile([128, 1], BF16, tag="ones_c")
        nc.gpsimd.memset(ones_c[:], 1.0)
        ones_r = const.tile([1, 128], BF16, tag="ones_r")
        nc.gpsimd.memset(ones_r[:], 1.0)

        # ---- memory preprocessing: norms, bf16 cast, transpose ----
        memS = [const.tile([128, C], BF16, tag=f"memS{mi}", name=f"memS{mi}")
                for mi in range(MC)]
        memT = [const.tile([128, M], BF16, tag=f"memT{ci}", name=f"memT{ci}")
                for ci in range(CC)]
        for mi in range(MC):
            mld = mstage.tile([128, C], F32)
            nc.sync.dma_start(mld[:], mem[mi * 128:(mi + 1) * 128, :])
            msq = mtmp.tile([128, C], F32, tag="msq")
            mssq = stats.tile([128, 1], F32, tag="mssq")
            nc.scalar.activation(msq[:], mld[:], AF.Square, accum_out=mssq[:])
            mnorm = stats.tile([128, 1], F32, tag="mnorm")
            nc.scalar.activation(mnorm[:], mssq[:], AF.Sqrt)
            rinv = stats.tile([128, 1], F32, tag="rinv")
            nc.vector.reciprocal(rinv[:], mnorm[:])
            nc.vector.tensor_copy(memS[mi][:], mld[:])
            mn = mtmp.tile([128, C], BF16, tag="mn")
            nc.vector.tensor_scalar_mul(mn[:], mld[:], rinv[:])
            for ci in range(CC):
                ptr = pbc.tile([128, 128], BF16, tag="ptr")
                nc.tensor.transpose(ptr[:], mn[:, ci * 128:(ci + 1) * 128],
                                    idb[:])
                nc.scalar.activation(
                    memT[ci][:, mi * 128:(mi + 1) * 128], ptr[:], AF.Copy)

        # ---- main loop: super-pairs of 512 pixels ----
        groups = [list(range(g, min(g + 2, ns_count)))
                  for g in range(0, ns_count, 2)]
        for b in range(b_loc):
            for grp in groups:
                xb, xsq = {}, {}
                for ns in grp:
                    for ci in range(CC):
                        xf = xio.tile([128, 512], F32, tag="xf")
                        nc.sync.dma_start(
                            xf[:], xs[b, ci * 128:(ci + 1) * 128,
                                      ns * 512:(ns + 1) * 512])
                        xbc = xio.tile([128, 512], BF16, tag="xb")
                        nc.scalar.activation(xbc[:], xf[:], AF.Copy)
                        xb[ns, ci] = xbc
                        xq = xio.tile([128, 512], BF16, tag="xsq")
                        nc.scalar.activation(xq[:], xf[:], AF.Square)
                        xsq[ns, ci] = xq
                # sumsq + threshold per super  -> mxp (masked max) per super
                thr = {}
                for ns in grp:
                    pq = psq.tile([1, 512], F32)
                    for ci in range(CC):
                        nc.tensor.matmul(pq[:], ones_c[:], xsq[ns, ci][:],
                                         start=(ci == 0), stop=(ci == CC - 1))
                    th = stats.tile([1, 512], F32, tag="thr")
                    nc.scalar.activation(th[:], pq[:], AF.Sqrt, scale=THRESH2)
                    thr[ns] = th
                # simT: 8 m-tiles per super; pair shares ldweights
                sT = {}
                for mt in range(MC):
                    ps = {}
                    for ci in range(CC):
                        for ns in grp:
                            if ci == 0:
                                ps[ns] = psim.tile([128, 512], F32,
                                                   tag="psim", name="ps")
                            nc.tensor.matmul(
                                ps[ns][:],
                                memT[ci][:, mt * 128:(mt + 1) * 128],
                                xb[ns, ci][:],
                                start=(ci == 0), stop=(ci == CC - 1))
                    for ns in grp:
                        st = simp.tile([128, 512], BF16, tag=f"sT{mt}",
                                       name=f"sT{mt}")
                        nc.scalar.activation(st[:], ps[ns][:], AF.Copy)
                        sT[ns, mt] = st
                oh = {}
                for ns in grp:
                    # column max over all 1024 memory rows
                    cm = stats.tile([128, 512], BF16, tag="cm")
                    nc.vector.tensor_tensor(cm[:], sT[ns, 0][:], sT[ns, 1][:],
                                            ALU.max)
                    for mt in range(2, MC):
                        nc.vector.tensor_tensor(cm[:], cm[:], sT[ns, mt][:],
                                                ALU.max)
                    cmB = stats.tile([128, 512], F32, tag="cmB")
                    nc.gpsimd.partition_all_reduce(cmB[:], cm[:], 128,
                                                   bass_isa.ReduceOp.max)
                    thrB = stats.tile([128, 512], F32, tag="thrB")
                    nc.gpsimd.partition_broadcast(thrB[:], thr[ns][:], 128)
                    # mask folded into the compare value:
                    # mxB = colmax + BIG * (colmax <= thr)
                    msk = stats.tile([128, 512], F32, tag="msk")
                    nc.vector.tensor_tensor(msk[:], cmB[:], thrB[:], ALU.is_le)
                    pen = stats.tile([128, 512], F32, tag="pen")
                    nc.vector.tensor_scalar_mul(pen[:], msk[:], BIG)
                    mxB = stats.tile([128, 512], BF16, tag="mxB")
                    nc.vector.tensor_tensor(mxB[:], cmB[:], pen[:], ALU.add)
                    for mt in range(MC):
                        o = simp.tile([128, 512], BF16, tag=f"oh{mt}",
                                      name=f"oh{mt}")
                        nc.vector.tensor_tensor(o[:], sT[ns, mt][:], mxB[:],
                                                ALU.is_equal)
                        oh[ns, mt] = o
                # phase B: out[c, n] = sum_m mem[m, c] * onehot[m, n]
                for ci in range(CC):
                    pB = {}
                    for mt in range(MC):
                        for ns in grp:
                            if mt == 0:
                                pB[ns] = psumB.tile([128, 512], F32,
                                                    tag="pB", name="pB")
                            nc.tensor.matmul(
                                pB[ns][:],
                                memS[mt][:, ci * 128:(ci + 1) * 128],
                                oh[ns, mt][:],
                                start=(mt == 0), stop=(mt == MC - 1))
                    for ns in grp:
                        ob = outb.tile([128, 512], F32, tag="ob")
                        nc.scalar.activation(ob[:], pB[ns][:], AF.Copy)
                        nc.sync.dma_start(
                            out[b, ci * 128:(ci + 1) * 128,
                                ns * 512:(ns + 1) * 512], ob[:])

    nc.compile()
    return nc


_NC_CACHE = {}


def _get_nc(b_loc=B_LOC, n_pix=N_PIX):
    key = (b_loc, n_pix)
    if key not in _NC_CACHE:
        _NC_CACHE[key] = build_kernel(*key)
    return _NC_CACHE[key]


def run_on_hw(x_flat, memory, b_loc=B_LOC, n_pix=N_PIX, trace=False,
              **spmd_kwargs):
    """x_flat: [N_CORES*b_loc, C, n_pix] f32. Returns (out_full, results)."""
    nc = _get_nc(b_loc, n_pix)
    ident_b = np.eye(128, dtype=ml_dtypes.bfloat16)
    in_maps = [
        {
            "xs": np.ascontiguousarray(x_flat[c * b_loc:(c + 1) * b_loc]),
            "memory": memory,
            "identity": ident_b,
        }
        for c in range(N_CORES)
    ]
    res = run_bass_kernel_spmd(nc, in_maps, list(range(N_CORES)),
                               trace=trace, **spmd_kwargs)
    outs = [res.results[c]["out"] for c in range(N_CORES)]
    return np.concatenate(outs, axis=0), res


def kernel(x, memory):
    x = np.asarray(x, dtype=np.float32)
    memory = np.asarray(memory, dtype=np.float32)
    B, C_, H_, W_ = x.shape
    x_flat = np.ascontiguousarray(x.reshape(B, C_, H_ * W_))
    out_flat, _ = run_on_hw(x_flat, memory)
    return out_flat.reshape(B, C_, H_, W_)


# revision 14
# speedup vs baseline: 1.7442x; 1.0173x over previous
"""HardMemory retrieval-KNN kernel for 8 Trainium2 NeuronCores.

Data-parallel: 32 batches sharded 4-per-core; memory bank [1024,512]
replicated. Per batch b (x_b = [C=512, N=4096] f32):
  simT[m,n]  = <x_n, mem_m / ||mem_m||>          (bf16 matmul, fp32 accum)
  sumsq[n]   = ||x_n||^2  (ones-stationary matmul over x^2)
  mx'[n]     = colmax(simT) + 1e30*(colmax <= 0.8*sqrt(sumsq))
  onehot[m,n]= (simT == bcast(mx'))               (bf16 {0,1}; mask folded in)
  out[:, n]  = memory^T @ onehot                  (masked gather as matmul)

simT layout avoids any onehot transpose; onehot feeds matmul2 directly
as the moving operand.
"""

import sys

for _p in ("/opt/trn_rl_repo",):
    if _p not in sys.path:
        sys.path.insert(0, _p)

from contextlib import ExitStack

import ml_dtypes
import numpy as np

import concourse.bass as bass
import concourse.tile as tile
from concourse import bacc, bass_isa, mybir
from concourse.bass_utils import run_bass_kernel_spmd

F32 = mybir.dt.float32
BF16 = mybir.dt.bfloat16
AF = mybir.ActivationFunctionType
ALU = mybir.AluOpType
AX = mybir.AxisListType

B_FULL, C, H, W = 32, 512, 64, 64
N_PIX = H * W
M = 1024
N_CORES = 8
B_LOC = B_FULL // N_CORES
THRESH2 = 0.8 * 0.8
BIG = 1.0e30

CC = C // 128            # 4 contraction chunks
MC = M // 128            # 8 memory chunks


def build_kernel(b_loc=B_LOC, n_pix=N_PIX):
    ns_count = n_pix // 512

    nc = bacc.Bacc("TRN2", target_bir_lowering=False, debug=False,
                   num_devices=N_CORES)
    xs = nc.dram_tensor("xs", [b_loc, C, n_pix], BF16, kind="ExternalInput")
    mem = nc.dram_tensor("memory", [M, C], F32, kind="ExternalInput")
    ident_b = nc.dram_tensor("identity", [128, 128], BF16, kind="ExternalInput")
    out = nc.dram_tensor("out", [b_loc, C, n_pix], F32, kind="ExternalOutput")

    with tile.TileContext(nc) as tc, ExitStack() as ctx:
        const = ctx.enter_context(tc.tile_pool(name="const", bufs=1))
        mstage = ctx.enter_context(tc.tile_pool(name="mstage", bufs=2))
        mtmp = ctx.enter_context(tc.tile_pool(name="mtmp", bufs=2))
        xio = ctx.enter_context(tc.tile_pool(name="xio", bufs=16))
        simp = ctx.enter_context(tc.tile_pool(name="simp", bufs=2))
        stats = ctx.enter_context(tc.tile_pool(name="stats", bufs=4))
        outb = ctx.enter_context(tc.tile_pool(name="outb", bufs=4))
        psim = ctx.enter_context(
            tc.tile_pool(name="psim", bufs=4, space=bass.MemorySpace.PSUM))
        psq = ctx.enter_context(
            tc.tile_pool(name="psq", bufs=1, space=bass.MemorySpace.PSUM))
        pbc = ctx.enter_context(
            tc.tile_pool(name="pbc", bufs=1, space=bass.MemorySpace.PSUM))
        psumB = ctx.enter_context(
            tc.tile_pool(name="psumB", bufs=2, space=bass.MemorySpace.PSUM))

        idb = const.tile([128, 128], BF16, tag="idb")
        nc.sync.dma_start(idb[:], ident_b[:])
        ones_c = const.tile([128, 1], BF16, tag="ones_c")
        nc.gpsimd.memset(ones_c[:], 1.0)
        ones_r = const.tile([1, 128], BF16, tag="ones_r")
        nc.gpsimd.memset(ones_r[:], 1.0)

        # ---- memory preprocessing: norms, bf16 cast, transpose ----
        memS = [const.tile([128, C], BF16, tag=f"memS{mi}", name=f"memS{mi}")
                for mi in range(MC)]
        memT = [const.tile([128, M], BF16, tag=f"memT{ci}", name=f"memT{ci}")
                for ci in range(CC)]
        for mi in range(MC):
            mld = mstage.tile([128, C], F32)
            nc.sync.dma_start(mld[:], mem[mi * 128:(mi + 1) * 128, :])
            msq = mtmp.tile([128, C], F32, tag="msq")
            mssq = stats.tile([128, 1], F32, tag="mssq")
            nc.scalar.activation(msq[:], mld[:], AF.Square, accum_out=mssq[:])
            mnorm = stats.tile([128, 1], F32, tag="mnorm")
            nc.scalar.activation(mnorm[:], mssq[:], AF.Sqrt)
            rinv = stats.tile([128, 1], F32, tag="rinv")
            nc.vector.reciprocal(rinv[:], mnorm[:])
            nc.vector.tensor_copy(memS[mi][:], mld[:])
            mn = mtmp.tile([128, C], BF16, tag="mn")
            nc.vector.tensor_scalar_mul(mn[:], mld[:], rinv[:])
            for ci in range(CC):
                ptr = pbc.tile([128, 128], BF16, tag="ptr")
                nc.tensor.transpose(ptr[:], mn[:, ci * 128:(ci + 1) * 128],
                                    idb[:])
                nc.scalar.activation(
                    memT[ci][:, mi * 128:(mi + 1) * 128], ptr[:], AF.Copy)

        # ---- main loop: super-pairs of 512 pixels ----
        groups = [list(range(g, min(g + 2, ns_count)))
                  for g in range(0, ns_count, 2)]
        for b in range(b_loc):
            for grp in groups:
                xb, xsq = {}, {}
                for ns in grp:
                    for ci in range(CC):
                        xbc = xio.tile([128, 512], BF16, tag="xb")
                        nc.sync.dma_start(
                            xbc[:], xs[b, ci * 128:(ci + 1) * 128,
                                       ns * 512:(ns + 1) * 512])
                        xb[ns, ci] = xbc
                        xq = xio.tile([128, 512], BF16, tag="xsq")
                        nc.vector.tensor_tensor(xq[:], xbc[:], xbc[:],
                                                ALU.mult)
                        xsq[ns, ci] = xq
                # sumsq + threshold per super  -> mxp (masked max) per super
                thr = {}
                for ns in grp:
                    pq = psq.tile([1, 512], F32)
                    for ci in range(CC):
                        nc.tensor.matmul(pq[:], ones_c[:], xsq[ns, ci][:],
                                         start=(ci == 0), stop=(ci == CC - 1))
                    th = stats.tile([1, 512], F32, tag="thr")
                    nc.scalar.activation(th[:], pq[:], AF.Sqrt, scale=THRESH2)
                    thr[ns] = th
                # simT: 8 m-tiles per super; pair shares ldweights
                sT = {}
                for mt in range(MC):
                    ps = {}
                    for ci in range(CC):
                        for ns in grp:
                            if ci == 0:
                                ps[ns] = psim.tile([128, 512], F32,
                                                   tag="psim", name="ps")
                            nc.tensor.matmul(
                                ps[ns][:],
                                memT[ci][:, mt * 128:(mt + 1) * 128],
                                xb[ns, ci][:],
                                start=(ci == 0), stop=(ci == CC - 1))
                    for ns in grp:
                        st = simp.tile([128, 512], BF16, tag=f"sT{mt}",
                                       name=f"sT{mt}")
                        nc.scalar.activation(st[:], ps[ns][:], AF.Copy)
                        sT[ns, mt] = st
                oh = {}
                for ns in grp:
                    # column max over all 1024 memory rows
                    cm = stats.tile([128, 512], BF16, tag="cm")
                    nc.vector.tensor_tensor(cm[:], sT[ns, 0][:], sT[ns, 1][:],
                                            ALU.max)
                    for mt in range(2, MC):
                        nc.vector.tensor_tensor(cm[:], cm[:], sT[ns, mt][:],
                                                ALU.max)
                    cmB = stats.tile([128, 512], F32, tag="cmB")
                    nc.gpsimd.partition_all_reduce(cmB[:], cm[:], 128,
                                                   bass_isa.ReduceOp.max)
                    thrB = stats.tile([128, 512], F32, tag="thrB")
                    nc.gpsimd.partition_broadcast(thrB[:], thr[ns][:], 128)
                    # mask folded into the compare value:
                    # mxB = colmax + BIG * (colmax <= thr)
                    msk = stats.tile([128, 512], F32, tag="msk")
                    nc.vector.tensor_tensor(msk[:], cmB[:], thrB[:], ALU.is_le)
                    pen = stats.tile([128, 512], F32, tag="pen")
                    nc.vector.tensor_scalar_mul(pen[:], msk[:], BIG)
                    mxB = stats.tile([128, 512], BF16, tag="mxB")
                    nc.vector.tensor_tensor(mxB[:], cmB[:], pen[:], ALU.add)
                    for mt in range(MC):
                        o = simp.tile([128, 512], BF16, tag=f"oh{mt}",
                                      name=f"oh{mt}")
                        nc.vector.tensor_tensor(o[:], sT[ns, mt][:], mxB[:],
                                                ALU.is_equal)
                        oh[ns, mt] = o
                # phase B: out[c, n] = sum_m mem[m, c] * onehot[m, n]
                for ci in range(CC):
                    pB = {}
                    for mt in range(MC):
                        for ns in grp:
                            if mt == 0:
                                pB[ns] = psumB.tile([128, 512], F32,
                                                    tag="pB", name="pB")
                            nc.tensor.matmul(
                                pB[ns][:],
                                memS[mt][:, ci * 128:(ci + 1) * 128],
                                oh[ns, mt][:],
                                start=(mt == 0), stop=(mt == MC - 1))
                    for ns in grp:
                        ob = outb.tile([128, 512], F32, tag="ob")
                        nc.scalar.activation(ob[:], pB[ns][:], AF.Copy)
                        nc.sync.dma_start(
                            out[b, ci * 128:(ci + 1) * 128,
                                ns * 512:(ns + 1) * 512], ob[:])

    nc.compile()
    return nc


_NC_CACHE = {}


def _get_nc(b_loc=B_LOC, n_pix=N_PIX):
    key = (b_loc, n_pix)
    if key not in _NC_CACHE:
        _NC_CACHE[key] = build_kernel(*key)
    return _NC_CACHE[key]


def run_on_hw(x_flat, memory, b_loc=B_LOC, n_pix=N_PIX, trace=False,
              **spmd_kwargs):
    """x_flat: [N_CORES*b_loc, C, n_pix] f32. Returns (out_full, results)."""
    nc = _get_nc(b_loc, n_pix)
    ident_b = np.eye(128, dtype=ml_dtypes.bfloat16)
    x_bf = x_flat.astype(ml_dtypes.bfloat16)
    in_maps = [
        {
            "xs": np.ascontiguousarray(x_bf[c * b_loc:(c + 1) * b_loc]),
            "memory": memory,
            "identity": ident_b,
        }
        for c in range(N_CORES)
    ]
    res = run_bass_kernel_spmd(nc, in_maps, list(range(N_CORES)),
                               trace=trace, **spmd_kwargs)
    outs = [res.results[c]["out"] for c in range(N_CORES)]
    return np.concatenate(outs, axis=0), res


def kernel(x, memory):
    x = np.asarray(x, dtype=np.float32)
    memory = np.asarray(memory, dtype=np.float32)
    B, C_, H_, W_ = x.shape
    x_flat = np.ascontiguousarray(x.reshape(B, C_, H_ * W_))
    out_flat, _ = run_on_hw(x_flat, memory)
    return out_flat.reshape(B, C_, H_, W_)
